# revision 4
# baseline (speedup 1.0000x reference)
"""Trainium2 Bass kernel for CompleteMOERTDETR MoE routing.

Math (per the reference):
  h = relu(features @ W1 + b1)                       [B,S,128]
  z = h @ W2 + b2                                    [B,S,6]
  p = softmax(z); top2 gating; w = softmax(top2 p)
  out_b[b,q] = sum_k w_k * expert_bboxes[idx_k, b, q]     (q < min(S,Q)=300)
  out_c[b,q] = sum_k w_k * expert_class_scores[idx_k, b, q]

Strategy: data-parallel over batch, 16 batches/core on 8 cores. The host
pre-transposes features to feature-major ([256, tokens]) and splits it into
bf16 hi/lo halves so mm1 runs as three bf16 matmuls (hi*Whi + hi*Wlo + lo*Whi),
which matches fp32 accuracy to ~1e-6 while streaming at 1 cycle/row. mm2 runs
in true fp32 with the relu'd h^T tile as the (self-loaded) stationary operand,
producing logits directly in token-partition layout. Top-2 selection uses a
bit-trick (clear low 3 mantissa bits of p=exp(z), OR in (5-e)) so the two
argmax masks are exactly one-hot even under ties, with ties broken toward the
lower expert index exactly like jax.lax.top_k. The combine is a dense
mask-weighted sum over the 6 experts done with a handful of wide DVE ops.
"""

import numpy as np
import ml_dtypes

# ---- problem constants (hardcoded per contest rules) ----
B, S, D, E, Q, C = 128, 400, 256, 6, 300, 6
NCORES = 8
BPC = B // NCORES              # 16 batches per core
NTOK = BPC * Q                 # 4800 live tokens per core
NT = 38                        # padded token tiles of 128
NTOKP = NT * 128               # 4864 padded tokens
NH = NT // 2                   # token tiles per half (19)
HALF = NH * 128                # tokens per half (2432)
CHUNKS = [512, 512, 512, 512, 384]   # per-half matmul chunks (sum = 2432)
DH = D // 2                    # 128

_PROG = None                   # cached compiled program


def _build_program():
    from contextlib import ExitStack
    import concourse.bacc as bacc
    import concourse.tile as tile
    from concourse import mybir

    f32 = mybir.dt.float32
    bf16 = mybir.dt.bfloat16
    u32 = mybir.dt.uint32
    A = mybir.AluOpType
    AF = mybir.ActivationFunctionType
    AX = mybir.AxisListType

    nc = bacc.Bacc("TRN2", target_bir_lowering=False, debug=False,
                   num_devices=NCORES)

    xhi = nc.dram_tensor("xhi", [D, NTOKP], bf16, kind="ExternalInput").ap()
    xlo = nc.dram_tensor("xlo", [D, NTOKP], bf16, kind="ExternalInput").ap()
    w1hi = nc.dram_tensor("w1hi", [D, DH], bf16, kind="ExternalInput").ap()
    w1lo = nc.dram_tensor("w1lo", [D, DH], bf16, kind="ExternalInput").ap()
    w2d = nc.dram_tensor("w2", [DH, E], f32, kind="ExternalInput").ap()
    b1d = nc.dram_tensor("b1", [DH, 1], f32, kind="ExternalInput").ap()
    b2d = nc.dram_tensor("b2bc", [128, 24], f32, kind="ExternalInput").ap()
    iord = nc.dram_tensor("iorc", [128, E], u32, kind="ExternalInput").ap()
    dbd = nc.dram_tensor("db", [128, NT * 24], f32, kind="ExternalInput").ap()
    dcd = nc.dram_tensor("dc", [128, NT * 36], f32, kind="ExternalInput").ap()
    outd = nc.dram_tensor("out", [128, NT * 10], f32, kind="ExternalOutput").ap()

    with tile.TileContext(nc) as tc, ExitStack() as ctx:
        consts = ctx.enter_context(tc.tile_pool(name="consts", bufs=1))
        xp = ctx.enter_context(tc.tile_pool(name="x", bufs=3))
        hp = ctx.enter_context(tc.tile_pool(name="h", bufs=3))
        php = ctx.enter_context(tc.tile_pool(name="ph", bufs=2, space="PSUM"))
        pzp = ctx.enter_context(tc.tile_pool(name="pz", bufs=2, space="PSUM"))
        big = ctx.enter_context(tc.tile_pool(name="big", bufs=1))

        # ---- constants ----
        w1hit = [consts.tile([128, DH], bf16, tag=f"w1hi{k}", name=f"w1hi{k}")
                 for k in range(2)]
        w1lot = [consts.tile([128, DH], bf16, tag=f"w1lo{k}", name=f"w1lo{k}")
                 for k in range(2)]
        for k in range(2):
            nc.sync.dma_start(w1hit[k][:], w1hi[k * 128:(k + 1) * 128, :])
            nc.sync.dma_start(w1lot[k][:], w1lo[k * 128:(k + 1) * 128, :])
        w2t = consts.tile([128, E], f32, tag="w2")
        nc.sync.dma_start(w2t[:], w2d[:])
        b1t = consts.tile([128, 1], f32, tag="b1")
        nc.sync.dma_start(b1t[:], b1d[:])
        b2t = consts.tile([128, 24], f32, tag="b2")
        nc.sync.dma_start(b2t[:], b2d[:])
        iorc = consts.tile([128, E], u32, tag="iorc")
        nc.sync.dma_start(iorc[:], iord[:])

        for half in range(2):
            htag = f"h{half}"
            tok0 = half * HALF

            db = big.tile([128, NH * 24], f32, tag=f"db{htag}")
            nc.sync.dma_start(db[:], dbd[:, half * NH * 24:(half + 1) * NH * 24])
            dc = big.tile([128, NH * 36], f32, tag=f"dc{htag}")
            nc.sync.dma_start(dc[:], dcd[:, half * NH * 36:(half + 1) * NH * 36])

            z = big.tile([128, NH * 6], f32, tag=f"z{htag}")

            # ---- phase A: router matmuls ----
            col = 0
            for W in CHUNKS:
                xt = []
                for k in range(2):
                    th = xp.tile([128, W], bf16, tag=f"xhi{k}")
                    nc.sync.dma_start(
                        th[:], xhi[k * 128:(k + 1) * 128, tok0 + col:tok0 + col + W])
                    tl = xp.tile([128, W], bf16, tag=f"xlo{k}")
                    nc.sync.dma_start(
                        tl[:], xlo[k * 128:(k + 1) * 128, tok0 + col:tok0 + col + W])
                    xt.append((th, tl))
                ph = php.tile([128, W], f32, tag="ph")
                mms = []
                for k in range(2):
                    th, tl = xt[k]
                    mms += [(w1hit[k], th), (w1lot[k], th), (w1hit[k], tl)]
                for i, (wt, mt) in enumerate(mms):
                    nc.tensor.matmul(ph[:], wt[:], mt[:],
                                     start=(i == 0), stop=(i == len(mms) - 1))
                hr = hp.tile([128, W], f32, tag="hr")
                nc.scalar.activation(hr[:], ph[:], AF.Relu, bias=b1t[:], scale=1.0)

                ntl = W // 128
                pz = pzp.tile([128, 24], f32, tag="pz")
                for t in range(ntl):
                    nc.tensor.matmul(pz[:, t * 6:(t + 1) * 6],
                                     hr[:, t * 128:(t + 1) * 128], w2t[:],
                                     start=(t == 0), stop=(t == ntl - 1))
                nc.vector.tensor_tensor(z[:, col // 128 * 6:col // 128 * 6 + ntl * 6],
                                        pz[:, :ntl * 6], b2t[:, :ntl * 6], op=A.add)
                col += W

            # ---- phase B: softmax / top2 / combine ----
            def v3(ap, e=6):
                return ap.rearrange("p (t e) -> p t e", e=e)

            p = big.tile([128, NH * 6], f32, tag=f"p{htag}")
            nc.scalar.activation(p[:], z[:], AF.Exp)
            s = big.tile([128, NH], f32, tag=f"s{htag}")
            nc.vector.tensor_reduce(s[:], v3(p[:]), axis=AX.X, op=A.add)
            r = big.tile([128, NH], f32, tag=f"r{htag}")
            nc.vector.reciprocal(r[:], s[:])

            qm = big.tile([128, NH * 6], u32, tag=f"qm{htag}")
            nc.vector.tensor_scalar(qm[:], p[:].bitcast(u32), 0xFFFFFFF8, None,
                                    op0=A.bitwise_and)
            q = big.tile([128, NH * 6], u32, tag=f"q{htag}")
            nc.vector.tensor_tensor(v3(q[:]), v3(qm[:]),
                                    iorc[:].unsqueeze(1).broadcast_to([128, NH, 6]),
                                    op=A.bitwise_or)
            qf = q[:].bitcast(f32)
            v1 = big.tile([128, NH], f32, tag=f"v1{htag}")
            nc.vector.tensor_reduce(v1[:], v3(qf), axis=AX.X, op=A.max)
            m1 = big.tile([128, NH * 6], f32, tag=f"m1{htag}")
            nc.vector.tensor_tensor(v3(m1[:]), v3(qf),
                                    v1[:].unsqueeze(2).broadcast_to([128, NH, 6]),
                                    op=A.is_equal)
            tmp = big.tile([128, NH * 6], f32, tag=f"tmp{htag}")
            nc.vector.tensor_scalar(tmp[:], m1[:], -1e30, None, op0=A.mult)
            pm = big.tile([128, NH * 6], f32, tag=f"pm{htag}")
            nc.vector.tensor_tensor(pm[:], qf, tmp[:], op=A.add)
            v2 = big.tile([128, NH], f32, tag=f"v2{htag}")
            nc.vector.tensor_reduce(v2[:], v3(pm[:]), axis=AX.X, op=A.max)
            m2 = big.tile([128, NH * 6], f32, tag=f"m2{htag}")
            nc.vector.tensor_tensor(v3(m2[:]), v3(pm[:]),
                                    v2[:].unsqueeze(2).broadcast_to([128, NH, 6]),
                                    op=A.is_equal)

            d_ = big.tile([128, NH], f32, tag=f"d{htag}")
            nc.vector.tensor_tensor(d_[:], v2[:], v1[:], op=A.subtract)
            t_ = big.tile([128, NH], f32, tag=f"t{htag}")
            nc.vector.tensor_tensor(t_[:], d_[:], r[:], op=A.mult)
            e2 = big.tile([128, NH], f32, tag=f"e2{htag}")
            nc.scalar.activation(e2[:], t_[:], AF.Exp)
            s2 = big.tile([128, NH], f32, tag=f"s2{htag}")
            nc.vector.tensor_scalar(s2[:], e2[:], 1.0, None, op0=A.add)
            w1_ = big.tile([128, NH], f32, tag=f"w1{htag}")
            nc.vector.reciprocal(w1_[:], s2[:])
            w2_ = big.tile([128, NH], f32, tag=f"w2{htag}")
            nc.vector.tensor_tensor(w2_[:], e2[:], w1_[:], op=A.mult)

            g1 = big.tile([128, NH * 6], f32, tag=f"g1{htag}")
            nc.vector.tensor_tensor(v3(g1[:]), v3(m1[:]),
                                    w1_[:].unsqueeze(2).broadcast_to([128, NH, 6]),
                                    op=A.mult)
            g2 = big.tile([128, NH * 6], f32, tag=f"g2{htag}")
            nc.vector.tensor_tensor(v3(g2[:]), v3(m2[:]),
                                    w2_[:].unsqueeze(2).broadcast_to([128, NH, 6]),
                                    op=A.mult)
            g = big.tile([128, NH * 6], f32, tag=f"g{htag}")
            nc.vector.tensor_tensor(g[:], g1[:], g2[:], op=A.add)

            def ved(ap, dd):
                return ap.rearrange("p (t e d) -> p t e d", e=6, d=dd)

            def vde(ap, dd):
                return ap.rearrange("p (t e d) -> p t d e", e=6, d=dd)

            g4 = v3(g[:]).unsqueeze(3).broadcast_to([128, NH, 6, 4])
            nc.vector.tensor_tensor(ved(db[:], 4), ved(db[:], 4), g4, op=A.mult)
            g6 = v3(g[:]).unsqueeze(3).broadcast_to([128, NH, 6, 6])
            nc.vector.tensor_tensor(ved(dc[:], 6), ved(dc[:], 6), g6, op=A.mult)

            oall = big.tile([128, NH * 10], f32, tag=f"o{htag}")
            o10 = oall[:].rearrange("p (t f) -> p t f", f=10)
            nc.vector.tensor_reduce(o10[:, :, 0:4], vde(db[:], 4), axis=AX.X, op=A.add)
            nc.vector.tensor_reduce(o10[:, :, 4:10], vde(dc[:], 6), axis=AX.X, op=A.add)
            nc.sync.dma_start(outd[:, half * NH * 10:(half + 1) * NH * 10], oall[:])

    nc.compile()
    return nc


def _get_program():
    global _PROG
    if _PROG is None:
        _PROG = _build_program()
    return _PROG


def _prep_core_inputs(features, W1, b1, W2, b2, db_all, dc_all, core):
    """Build the per-core input map (all numpy)."""
    b0 = core * BPC
    # features slice -> [tokens, D] -> transposed [D, tokens], padded
    x = np.ascontiguousarray(
        features[b0:b0 + BPC, :Q, :].reshape(NTOK, D).T)          # [D, NTOK] f32
    xt = np.zeros((D, NTOKP), np.float32)
    xt[:, :NTOK] = x
    xhi = xt.astype(ml_dtypes.bfloat16)
    xlo = (xt - xhi.astype(np.float32)).astype(ml_dtypes.bfloat16)

    return {
        "xhi": xhi,
        "xlo": xlo,
        "w1hi": _W1HI,
        "w1lo": _W1LO,
        "w2": np.ascontiguousarray(W2.astype(np.float32)),
        "b1": np.ascontiguousarray(b1.astype(np.float32).reshape(DH, 1)),
        "b2bc": np.ascontiguousarray(
            np.broadcast_to(np.tile(b2.astype(np.float32), 4), (128, 24))),
        "iorc": np.ascontiguousarray(
            np.broadcast_to(5 - np.arange(E, dtype=np.uint32), (128, E))),
        "db": db_all[core],
        "dc": dc_all[core],
    }


_W1HI = None
_W1LO = None


def _prep_expert(data, dd):
    """[E,B,Q,dd] -> per-core [128, NT*E*dd] in (p, t, e, d) token-tile order."""
    # -> [B, Q, E, dd] -> [B*Q, E*dd]
    flat = np.ascontiguousarray(np.transpose(data, (1, 2, 0, 3))).reshape(B * Q, E * dd)
    out = []
    for core in range(NCORES):
        sl = flat[core * NTOK:(core + 1) * NTOK]
        pad = np.zeros((NTOKP, E * dd), np.float32)
        pad[:NTOK] = sl
        # token = t*128 + p  ->  [NT, 128, E*dd] -> [128, NT, E*dd]
        arr = pad.reshape(NT, 128, E * dd).transpose(1, 0, 2)
        out.append(np.ascontiguousarray(arr.reshape(128, NT * E * dd)))
    return out


def kernel(features, W1, b1, W2, b2, expert_bboxes, expert_class_scores):
    global _W1HI, _W1LO
    features = np.asarray(features, np.float32)
    W1 = np.asarray(W1, np.float32)
    b1 = np.asarray(b1, np.float32)
    W2 = np.asarray(W2, np.float32)
    b2 = np.asarray(b2, np.float32)
    expert_bboxes = np.asarray(expert_bboxes, np.float32)
    expert_class_scores = np.asarray(expert_class_scores, np.float32)

    _W1HI = W1.astype(ml_dtypes.bfloat16)
    _W1LO = (W1 - _W1HI.astype(np.float32)).astype(ml_dtypes.bfloat16)

    db_all = _prep_expert(expert_bboxes, 4)
    dc_all = _prep_expert(expert_class_scores, C)

    in_maps = [
        _prep_core_inputs(features, W1, b1, W2, b2, db_all, dc_all, core)
        for core in range(NCORES)
    ]

    global _LAST_IN_MAPS
    _LAST_IN_MAPS = in_maps

    nc = _get_program()
    from concourse.bass_utils import run_bass_kernel_spmd
    res = run_bass_kernel_spmd(nc, in_maps, list(range(NCORES)))

    bb = np.zeros((B, Q, 4), np.float32)
    cc = np.zeros((B, Q, C), np.float32)
    for core in range(NCORES):
        o = res.results[core]["out"]                      # [128, NT*10]
        o = o.reshape(128, NT, 10).transpose(1, 0, 2).reshape(NTOKP, 10)[:NTOK]
        b0 = core * BPC
        bb[b0:b0 + BPC] = o[:, 0:4].reshape(BPC, Q, 4)
        cc[b0:b0 + BPC] = o[:, 4:10].reshape(BPC, Q, C)
    return bb, cc


# revision 5
# speedup vs baseline: 1.0429x; 1.0429x over previous
"""Trainium2 Bass kernel for CompleteMOERTDETR MoE routing.

Reference math:
  h = relu(features @ W1 + b1); z = h @ W2 + b2; p = softmax(z)
  top-2 gating with renormalizing softmax; weighted combine of per-expert
  bboxes / class scores over the first min(S,Q)=300 positions.

Design (data-parallel over batch, 16 batches x 300 queries = 4800 tokens/core,
padded to 5120 = 40 token-tiles, on 8 cores):
  * Host pre-transposes features to feature-major [256, tokens] and splits
    into exact bf16 hi/lo halves; mm1 = hi@W1hi + hi@W1lo + lo@W1hi runs as
    six bf16 matmuls per 512-token chunk (1 cycle/row) with fp32-level
    accuracy (~1e-6), validated to preserve every top-2 selection.
  * relu on ACT (PSUM->SBUF) with per-partition bias b1.
  * mm2 is W2-stationary true fp32 (exact): each 512-token chunk writes
    z^T into a 32-row partition group of a shared PSUM bank (W2 zero-padded
    to 32 cols so the full group is written); one DVE copy + four PE
    transposes per 4-chunk super-group return z to token-partition layout.
  * top-2 selection via bit-trick: q = (exp(z) & ~7) | (5-e) makes argmax
    masks exactly one-hot with ties broken toward the lower expert index,
    matching jax.lax.top_k. Weights w1 = 1/(1+e^t), w2 = e^t*w1 with
    t = (v2-v1)/sum(p) reproduce softmax-of-top2 exactly.
  * combine = dense mask-weighted sum over E=6 with wide DVE ops on
    host-prepacked [p, tile, e, d] expert tables.
"""

import numpy as np
import ml_dtypes

# ---- problem constants (hardcoded per contest rules) ----
B, S, D, E, Q, C = 128, 400, 256, 6, 300, 6
NCORES = 8
BPC = B // NCORES              # 16 batches per core
NTOK = BPC * Q                 # 4800 live tokens per core
NT = 40                        # padded token tiles of 128
NTOKP = NT * 128               # 5120 padded tokens
NCHUNK = 10                    # 512-token matmul chunks
SGS = [4, 4, 2]                # super-groups of chunks (phase-B granularity)
DH = D // 2                    # 128

_PROG = None
_LAST_IN_MAPS = None


def _build_program():
    from contextlib import ExitStack
    import concourse.bacc as bacc
    import concourse.tile as tile
    from concourse import mybir

    f32 = mybir.dt.float32
    bf16 = mybir.dt.bfloat16
    u32 = mybir.dt.uint32
    A = mybir.AluOpType
    AF = mybir.ActivationFunctionType
    AX = mybir.AxisListType

    nc = bacc.Bacc("TRN2", target_bir_lowering=False, debug=False,
                   num_devices=NCORES)

    xpk = nc.dram_tensor("xpk", [D, 2, NTOKP], bf16, kind="ExternalInput").ap()
    cb16 = nc.dram_tensor("cb16", [128, 512], bf16, kind="ExternalInput").ap()
    cf32 = nc.dram_tensor("cf32", [128, 167], f32, kind="ExternalInput").ap()
    cu32 = nc.dram_tensor("cu32", [128, E], u32, kind="ExternalInput").ap()
    dbd = nc.dram_tensor("db", [128, NT * 24], f32, kind="ExternalInput").ap()
    dcd = nc.dram_tensor("dc", [128, NT * 36], f32, kind="ExternalInput").ap()
    outd = nc.dram_tensor("out", [128, NT * 10], f32, kind="ExternalOutput").ap()

    with tile.TileContext(nc) as tc, ExitStack() as ctx:
        consts = ctx.enter_context(tc.tile_pool(name="consts", bufs=1))
        xp = ctx.enter_context(tc.tile_pool(name="x", bufs=2))
        hp = ctx.enter_context(tc.tile_pool(name="h", bufs=3))
        ztsp = ctx.enter_context(tc.tile_pool(name="zts", bufs=2))
        php = ctx.enter_context(tc.tile_pool(name="ph", bufs=2, space="PSUM"))
        pzp = ctx.enter_context(tc.tile_pool(name="pz", bufs=2, space="PSUM"))
        ptp = ctx.enter_context(tc.tile_pool(name="pt", bufs=2, space="PSUM"))
        big = ctx.enter_context(tc.tile_pool(name="big", bufs=1))

        # ---- constant loads ----
        wb = consts.tile([128, 512], bf16, tag="wb")
        nc.sync.dma_start(wb[:], cb16[:])
        # first x super-DMA (1024 tokens, both kc) before remaining consts
        xt0 = [None, None]
        for k in range(2):
            t = xp.tile([128, 2048], bf16, tag=f"x{k}", name=f"x{k}g0")
            nc.sync.dma_start(t[:], xpk[k * 128:(k + 1) * 128, :, 0:1024])
            xt0[k] = t
        cf = consts.tile([128, 167], f32, tag="cf")
        nc.sync.dma_start(cf[:], cf32[:])
        iorc = consts.tile([128, E], u32, tag="iorc")
        nc.sync.dma_start(iorc[:], cu32[:])

        w1hi = [wb[:, 0:128], wb[:, 256:384]]
        w1lo = [wb[:, 128:256], wb[:, 384:512]]
        w2t = cf[:, 0:32]
        b1t = cf[:, 32:33]
        ident = cf[:, 33:161]
        b2t = cf[:, 161:167]

        xtiles = {0: xt0}
        sg_of_chunk = []
        for sgi, n in enumerate(SGS):
            sg_of_chunk += [sgi] * n
        sg_start = [0, 4, 8]

        db = [None] * 3
        dc = [None] * 3
        hr_of_chunk = [None] * NCHUNK
        ztp = [None] * 3
        tile_rng = [(0, 16), (16, 32), (32, 40)]

        def emit_sg_tail(sgi):
            """zts copy + transposes + z copy for super-group sgi."""
            n = SGS[sgi]
            nparts = 32 * n
            zts = ztsp.tile([128, 512], f32, tag="zts", name=f"zts{sgi}")
            nc.vector.tensor_copy(zts[0:nparts, :], ztp[sgi][0:nparts, :])
            ztr = ptp.tile([128, 512], f32, tag="ztr", name=f"ztr{sgi}")
            for b in range(4):
                nc.tensor.matmul(ztr[:, b * 128:(b + 1) * 128],
                                 zts[:, b * 128:(b + 1) * 128], ident,
                                 is_transpose=True, start=True, stop=True,
                                 skip_group_check=True)
            # ztr col = 128*b + 32*c + e ; local tile t = n*b... tile index
            # within sg: token = sgstart*512 + c*512 + b*128 + p -> tile = 4c+b
            t0, t1 = tile_rng[sgi]
            ntg = t1 - t0
            z = big.tile([128, ntg * 6], f32, tag=f"z{sgi}", name=f"z{sgi}")
            src = ztr[:].rearrange("p (b c e) -> p b c e", b=4, c=4)[:, :, 0:n, 0:E]
            dst = z[:].rearrange("p (c b e) -> p b c e", c=n, b=4)
            nc.vector.tensor_tensor(
                dst, src,
                b2t.unsqueeze(1).unsqueeze(2).broadcast_to([128, 4, n, E]),
                op=A.add)
            return z

        def emit_phase_b(sgi, z):
            t0, t1 = tile_rng[sgi]
            ntg = t1 - t0
            ZW = ntg * 6

            def v3(ap):
                return ap.rearrange("p (t e) -> p t e", e=6)

            def mk(wid, dt_=f32, nm=""):
                return big.tile([128, wid], dt_, tag=f"{nm}{sgi}",
                                name=f"{nm}{sgi}")

            p = mk(ZW, nm="p")
            nc.scalar.activation(p[:], z[:], AF.Exp)
            s = mk(ntg, nm="s")
            nc.vector.tensor_reduce(s[:], v3(p[:]), axis=AX.X, op=A.add)
            r = mk(ntg, nm="r")
            nc.vector.reciprocal(r[:], s[:])
            qm = mk(ZW, u32, nm="qm")
            nc.vector.tensor_scalar(qm[:], p[:].bitcast(u32), 0xFFFFFFF8, None,
                                    op0=A.bitwise_and)
            q = mk(ZW, u32, nm="q")
            nc.vector.tensor_tensor(v3(q[:]), v3(qm[:]),
                                    iorc[:].unsqueeze(1).broadcast_to([128, ntg, 6]),
                                    op=A.bitwise_or)
            qf = q[:].bitcast(f32)
            v1 = mk(ntg, nm="v1")
            nc.vector.tensor_reduce(v1[:], v3(qf), axis=AX.X, op=A.max)
            m1 = mk(ZW, nm="m1")
            nc.vector.tensor_tensor(v3(m1[:]), v3(qf),
                                    v1[:].unsqueeze(2).broadcast_to([128, ntg, 6]),
                                    op=A.is_equal)
            tmp = mk(ZW, nm="tmp")
            nc.vector.tensor_scalar(tmp[:], m1[:], -1e30, None, op0=A.mult)
            pm = mk(ZW, nm="pm")
            nc.vector.tensor_tensor(pm[:], qf, tmp[:], op=A.add)
            v2 = mk(ntg, nm="v2")
            nc.vector.tensor_reduce(v2[:], v3(pm[:]), axis=AX.X, op=A.max)
            m2 = mk(ZW, nm="m2")
            nc.vector.tensor_tensor(v3(m2[:]), v3(pm[:]),
                                    v2[:].unsqueeze(2).broadcast_to([128, ntg, 6]),
                                    op=A.is_equal)
            d_ = mk(ntg, nm="d")
            nc.vector.tensor_tensor(d_[:], v2[:], v1[:], op=A.subtract)
            t_ = mk(ntg, nm="t")
            nc.vector.tensor_tensor(t_[:], d_[:], r[:], op=A.mult)
            e2 = mk(ntg, nm="e2")
            nc.scalar.activation(e2[:], t_[:], AF.Exp)
            s2 = mk(ntg, nm="s2")
            nc.vector.tensor_scalar(s2[:], e2[:], 1.0, None, op0=A.add)
            w1_ = mk(ntg, nm="w1")
            nc.vector.reciprocal(w1_[:], s2[:])
            w2_ = mk(ntg, nm="w2")
            nc.vector.tensor_tensor(w2_[:], e2[:], w1_[:], op=A.mult)
            g1 = mk(ZW, nm="g1")
            nc.vector.tensor_tensor(v3(g1[:]), v3(m1[:]),
                                    w1_[:].unsqueeze(2).broadcast_to([128, ntg, 6]),
                                    op=A.mult)
            g = mk(ZW, nm="g")
            nc.vector.tensor_tensor(v3(g[:]), v3(m2[:]),
                                    w2_[:].unsqueeze(2).broadcast_to([128, ntg, 6]),
                                    op=A.mult)
            nc.vector.tensor_tensor(g[:], g[:], g1[:], op=A.add)

            def ved(ap, dd):
                return ap.rearrange("p (t e d) -> p t e d", e=6, d=dd)

            def vde(ap, dd):
                return ap.rearrange("p (t e d) -> p t d e", e=6, d=dd)

            g4 = v3(g[:]).unsqueeze(3).broadcast_to([128, ntg, 6, 4])
            nc.vector.tensor_tensor(ved(db[sgi][:], 4), ved(db[sgi][:], 4),
                                    g4, op=A.mult)
            g6 = v3(g[:]).unsqueeze(3).broadcast_to([128, ntg, 6, 6])
            nc.vector.tensor_tensor(ved(dc[sgi][:], 6), ved(dc[sgi][:], 6),
                                    g6, op=A.mult)
            oall = mk(ntg * 10, nm="o")
            o10 = oall[:].rearrange("p (t f) -> p t f", f=10)
            nc.vector.tensor_reduce(o10[:, :, 0:4], vde(db[sgi][:], 4),
                                    axis=AX.X, op=A.add)
            nc.vector.tensor_reduce(o10[:, :, 4:10], vde(dc[sgi][:], 6),
                                    axis=AX.X, op=A.add)
            nc.gpsimd.dma_start(outd[:, t0 * 10:t1 * 10], oall[:])

        # ---- main chunk loop ----
        for c in range(NCHUNK):
            sgi = sg_of_chunk[c]
            g = c // 2
            if c % 2 == 0 and g > 0:
                xt = [None, None]
                for k in range(2):
                    t = xp.tile([128, 2048], bf16, tag=f"x{k}", name=f"x{k}g{g}")
                    nc.sync.dma_start(
                        t[:], xpk[k * 128:(k + 1) * 128, :,
                                  g * 1024:(g + 1) * 1024])
                    xt[k] = t
                xtiles[g] = xt
            if c == 2:
                for i in range(3):
                    t0, t1 = tile_rng[i]
                    db[i] = big.tile([128, (t1 - t0) * 24], f32, tag=f"db{i}",
                                     name=f"db{i}")
                    nc.gpsimd.dma_start(db[i][:], dbd[:, t0 * 24:t1 * 24])
                    dc[i] = big.tile([128, (t1 - t0) * 36], f32, tag=f"dc{i}",
                                     name=f"dc{i}")
                    nc.gpsimd.dma_start(dc[i][:], dcd[:, t0 * 36:t1 * 36])

            off = (c % 2) * 512
            ph = php.tile([128, 512], f32, tag="ph", name=f"ph{c}")
            mms = []
            for k in range(2):
                hi = xtiles[g][k][:, off:off + 512]
                lo = xtiles[g][k][:, 1024 + off:1024 + off + 512]
                mms += [(w1hi[k], hi), (w1hi[k], lo), (w1lo[k], hi)]
            for i, (wt, mt) in enumerate(mms):
                nc.tensor.matmul(ph[:], wt, mt,
                                 start=(i == 0), stop=(i == len(mms) - 1))
            hr = hp.tile([128, 512], f32, tag="hr", name=f"hr{c}")
            nc.scalar.activation(hr[:], ph[:], AF.Relu, bias=b1t, scale=1.0)
            hr_of_chunk[c] = hr

            if c == sg_start[sgi]:
                ztp[sgi] = pzp.tile([128, 512], f32, tag="ztp", name=f"ztp{sgi}")
            cg = c - sg_start[sgi]
            nc.tensor.matmul(ztp[sgi][32 * cg:32 * cg + 32, :],
                             w2t, hr[:],
                             start=True, stop=True,
                             tile_position=(0, 32 * cg),
                             skip_group_check=True)

            if c == sg_start[sgi] + SGS[sgi] - 1:
                z = emit_sg_tail(sgi)
                emit_phase_b(sgi, z)

    nc.compile()
    return nc


def _get_program():
    global _PROG
    if _PROG is None:
        _PROG = _build_program()
    return _PROG


def _prep_expert(data, dd):
    """[E,B,Q,dd] -> per-core [128, NT*E*dd] in (p, t, e, d) token-tile order."""
    flat = np.ascontiguousarray(np.transpose(data, (1, 2, 0, 3))).reshape(B * Q, E * dd)
    out = []
    for core in range(NCORES):
        sl = flat[core * NTOK:(core + 1) * NTOK]
        pad = np.zeros((NTOKP, E * dd), np.float32)
        pad[:NTOK] = sl
        arr = pad.reshape(NT, 128, E * dd).transpose(1, 0, 2)
        out.append(np.ascontiguousarray(arr.reshape(128, NT * E * dd)))
    return out


def kernel(features, W1, b1, W2, b2, expert_bboxes, expert_class_scores):
    global _LAST_IN_MAPS
    features = np.asarray(features, np.float32)
    W1 = np.asarray(W1, np.float32)
    b1 = np.asarray(b1, np.float32)
    W2 = np.asarray(W2, np.float32)
    b2 = np.asarray(b2, np.float32)
    expert_bboxes = np.asarray(expert_bboxes, np.float32)
    expert_class_scores = np.asarray(expert_class_scores, np.float32)

    w1hi = W1.astype(ml_dtypes.bfloat16)
    w1lo = (W1 - w1hi.astype(np.float32)).astype(ml_dtypes.bfloat16)
    cb16 = np.zeros((128, 512), ml_dtypes.bfloat16)
    for k in range(2):
        cb16[:, k * 256:k * 256 + 128] = w1hi[k * 128:(k + 1) * 128, :]
        cb16[:, k * 256 + 128:k * 256 + 256] = w1lo[k * 128:(k + 1) * 128, :]

    cf32 = np.zeros((128, 167), np.float32)
    cf32[:, 0:E] = W2
    cf32[:, 32] = b1
    cf32[:, 33:161] = np.eye(128, dtype=np.float32)
    cf32[:, 161:167] = b2[None, :]

    cu32 = np.ascontiguousarray(
        np.broadcast_to(5 - np.arange(E, dtype=np.uint32), (128, E)))

    db_all = _prep_expert(expert_bboxes, 4)
    dc_all = _prep_expert(expert_class_scores, C)

    in_maps = []
    for core in range(NCORES):
        b0 = core * BPC
        x = np.ascontiguousarray(
            features[b0:b0 + BPC, :Q, :].reshape(NTOK, D).T)
        xt = np.zeros((D, NTOKP), np.float32)
        xt[:, :NTOK] = x
        xhi = xt.astype(ml_dtypes.bfloat16)
        xlo = (xt - xhi.astype(np.float32)).astype(ml_dtypes.bfloat16)
        xpk = np.stack([xhi, xlo], axis=1)        # [256, 2, NTOKP]
        in_maps.append({
            "xpk": np.ascontiguousarray(xpk),
            "cb16": cb16, "cf32": cf32, "cu32": cu32,
            "db": db_all[core], "dc": dc_all[core],
        })
    _LAST_IN_MAPS = in_maps

    nc = _get_program()
    from concourse.bass_utils import run_bass_kernel_spmd
    res = run_bass_kernel_spmd(nc, in_maps, list(range(NCORES)))

    bb = np.zeros((B, Q, 4), np.float32)
    cc = np.zeros((B, Q, C), np.float32)
    for core in range(NCORES):
        o = res.results[core]["out"]
        o = o.reshape(128, NT, 10).transpose(1, 0, 2).reshape(NTOKP, 10)[:NTOK]
        b0 = core * BPC
        bb[b0:b0 + BPC] = o[:, 0:4].reshape(BPC, Q, 4)
        cc[b0:b0 + BPC] = o[:, 4:10].reshape(BPC, Q, C)
    return bb, cc


# revision 7
# speedup vs baseline: 1.1054x; 1.0600x over previous
"""Trainium2 Bass kernel for CompleteMOERTDETR MoE routing.

Reference math:
  h = relu(features @ W1 + b1); z = h @ W2 + b2; p = softmax(z)
  top-2 gating with renormalizing softmax; weighted combine of per-expert
  bboxes / class scores over the first min(S,Q)=300 positions.

Design (data-parallel over batch, 16 batches x 300 queries = 4800 tokens/core,
padded to 5120 = 40 token-tiles, on 8 cores):
  * Host pre-transposes features to feature-major [256, tokens] and splits
    into exact bf16 hi/lo halves; mm1 = hi@W1hi + hi@W1lo + lo@W1hi runs as
    six bf16 matmuls per 512-token chunk (1 cycle/row) with fp32-level
    accuracy (~1e-6), validated to preserve every top-2 selection.
  * relu on ACT (PSUM->SBUF) with per-partition bias b1.
  * mm2 is W2-stationary true fp32 (exact): each 512-token chunk writes
    z^T into a 32-row partition group of a shared PSUM bank (W2 zero-padded
    to 32 cols so the full group is written); one DVE copy + four PE
    transposes per 4-chunk super-group return z to token-partition layout.
  * top-2 selection via bit-trick: q = (exp(z) & ~7) | (5-e) makes argmax
    masks exactly one-hot with ties broken toward the lower expert index,
    matching jax.lax.top_k. Weights w1 = 1/(1+e^t), w2 = e^t*w1 with
    t = (v2-v1)/sum(p) reproduce softmax-of-top2 exactly.
  * combine = dense mask-weighted sum over E=6 with wide DVE ops on
    host-prepacked [p, tile, e, d] expert tables.
"""

import numpy as np
import ml_dtypes

# ---- problem constants (hardcoded per contest rules) ----
B, S, D, E, Q, C = 128, 400, 256, 6, 300, 6
NCORES = 8
BPC = B // NCORES              # 16 batches per core
NTOK = BPC * Q                 # 4800 live tokens per core
NT = 38                        # padded token tiles of 128
NTOKP = NT * 128               # 4864 padded tokens
CHUNKW = [512] * 9 + [256]     # matmul chunk widths
SGS = [4, 4, 1, 1]             # super-groups of chunks (phase-B granularity)
NGRP = 5                       # x-DMA groups (1024 tokens each, last 768)
GRPW = [1024, 1024, 1024, 1024, 768]
DH = D // 2                    # 128

_PROG = None
_LAST_IN_MAPS = None


def _build_program():
    from contextlib import ExitStack
    import concourse.bacc as bacc
    import concourse.tile as tile
    from concourse import mybir

    f32 = mybir.dt.float32
    bf16 = mybir.dt.bfloat16
    u32 = mybir.dt.uint32
    A = mybir.AluOpType
    AF = mybir.ActivationFunctionType
    AX = mybir.AxisListType

    nc = bacc.Bacc("TRN2", target_bir_lowering=False, debug=False,
                   num_devices=NCORES)

    xpk = nc.dram_tensor("xpk", [D, NGRP, 2, 1024], bf16, kind="ExternalInput").ap()
    cb16 = nc.dram_tensor("cb16", [128, 512], bf16, kind="ExternalInput").ap()
    cf32 = nc.dram_tensor("cf32", [128, 167], f32, kind="ExternalInput").ap()
    cu32 = nc.dram_tensor("cu32", [128, E], u32, kind="ExternalInput").ap()
    dbd = nc.dram_tensor("db", [128, NT * 24], f32, kind="ExternalInput").ap()
    dcd = nc.dram_tensor("dc", [128, NT * 36], f32, kind="ExternalInput").ap()
    outd = nc.dram_tensor("out", [128, NT * 10], f32, kind="ExternalOutput").ap()

    with tile.TileContext(nc) as tc, ExitStack() as ctx:
        consts = ctx.enter_context(tc.tile_pool(name="consts", bufs=1))
        xp = ctx.enter_context(tc.tile_pool(name="x", bufs=2))
        hp = ctx.enter_context(tc.tile_pool(name="h", bufs=3))
        ztsp = ctx.enter_context(tc.tile_pool(name="zts", bufs=2))
        php = ctx.enter_context(tc.tile_pool(name="ph", bufs=2, space="PSUM"))
        pzp = ctx.enter_context(tc.tile_pool(name="pz", bufs=2, space="PSUM"))
        ptp = ctx.enter_context(tc.tile_pool(name="pt", bufs=2, space="PSUM"))
        big = ctx.enter_context(tc.tile_pool(name="big", bufs=1))

        # ---- constant loads ----
        wb = consts.tile([128, 512], bf16, tag="wb")
        nc.sync.dma_start(wb[:], cb16[:])
        # first x super-DMA (1024 tokens, both kc) before remaining consts
        xt0 = [None, None]
        for k in range(2):
            t = xp.tile([128, 2048], bf16, tag=f"x{k}", name=f"x{k}g0")
            nc.sync.dma_start(t[:], xpk[k * 128:(k + 1) * 128, 0, :, :])
            xt0[k] = t
        cf = consts.tile([128, 167], f32, tag="cf")
        nc.sync.dma_start(cf[:], cf32[:])
        iorc = consts.tile([128, E], u32, tag="iorc")
        nc.sync.dma_start(iorc[:], cu32[:])

        w1hi = [wb[:, 0:128], wb[:, 256:384]]
        w1lo = [wb[:, 128:256], wb[:, 384:512]]
        w2t = cf[:, 0:32]
        b1t = cf[:, 32:33]
        ident = cf[:, 33:161]
        b2t = cf[:, 161:167]

        xtiles = {0: xt0}
        sg_of_chunk = []
        for sgi, n in enumerate(SGS):
            sg_of_chunk += [sgi] * n
        sg_start = [0, 4, 8, 9]
        NCHUNK = len(sg_of_chunk)
        NSG = len(SGS)

        db = [None] * NSG
        dc = [None] * NSG
        ztp = [None] * NSG
        tile_rng = [(0, 16), (16, 32), (32, 36), (36, 38)]

        # PE warm-up: dense dummy matmuls so HAM unthrottles before real work
        wpsum = ptp.tile([128, 128], f32, tag="warm", name="warm")
        for wi in range(40):
            nc.tensor.matmul(wpsum[:], wb[:, 0:128], wb[:, 0:128],
                             start=True, stop=True, skip_group_check=True)

        def emit_sg_tail(sgi, wid):
            """zts copy + transposes + z copy for super-group sgi."""
            n = SGS[sgi]
            nb = wid // 128
            nparts = 32 * n
            zts = ztsp.tile([128, 512], f32, tag="zts", name=f"zts{sgi}")
            nc.scalar.copy(zts[0:nparts, 0:wid], ztp[sgi][0:nparts, 0:wid])
            ztr = ptp.tile([128, 512], f32, tag="ztr", name=f"ztr{sgi}")
            for b in range(nb):
                nc.tensor.matmul(ztr[:, b * 128:(b + 1) * 128],
                                 zts[:, b * 128:(b + 1) * 128], ident,
                                 is_transpose=True, start=True, stop=True,
                                 skip_group_check=True)
            # ztr col = 128*b + 32*c + e ; tile within sg = nb*c + b
            t0, t1 = tile_rng[sgi]
            ntg = t1 - t0
            z = big.tile([128, ntg * 6], f32, tag=f"z{sgi}", name=f"z{sgi}")
            src = ztr[:, 0:nb * 128].rearrange("p (b c e) -> p b c e",
                                               b=nb, c=4)[:, :, 0:n, 0:E]
            dst = z[:].rearrange("p (c b e) -> p b c e", c=n, b=nb)
            nc.vector.tensor_tensor(
                dst, src,
                b2t.unsqueeze(1).unsqueeze(2).broadcast_to([128, nb, n, E]),
                op=A.add)
            return z

        def emit_phase_b(sgi, z):
            t0, t1 = tile_rng[sgi]
            ntg = t1 - t0
            ZW = ntg * 6

            def v3(ap):
                return ap.rearrange("p (t e) -> p t e", e=6)

            def mk(wid, dt_=f32, nm=""):
                return big.tile([128, wid], dt_, tag=f"{nm}{sgi}",
                                name=f"{nm}{sgi}")

            p = mk(ZW, nm="p")
            nc.scalar.activation(p[:], z[:], AF.Exp)
            s = mk(ntg, nm="s")
            nc.vector.tensor_reduce(s[:], v3(p[:]), axis=AX.X, op=A.add)
            r = mk(ntg, nm="r")
            nc.vector.reciprocal(r[:], s[:])
            qm = mk(ZW, u32, nm="qm")
            nc.vector.tensor_scalar(qm[:], p[:].bitcast(u32), 0xFFFFFFF8, None,
                                    op0=A.bitwise_and)
            q = mk(ZW, u32, nm="q")
            nc.vector.tensor_tensor(v3(q[:]), v3(qm[:]),
                                    iorc[:].unsqueeze(1).broadcast_to([128, ntg, 6]),
                                    op=A.bitwise_or)
            qf = q[:].bitcast(f32)
            v1 = mk(ntg, nm="v1")
            nc.vector.tensor_reduce(v1[:], v3(qf), axis=AX.X, op=A.max)
            m1 = mk(ZW, nm="m1")
            nc.vector.tensor_tensor(v3(m1[:]), v3(qf),
                                    v1[:].unsqueeze(2).broadcast_to([128, ntg, 6]),
                                    op=A.is_equal)
            tmp = mk(ZW, nm="tmp")
            nc.vector.tensor_scalar(tmp[:], m1[:], -1e30, None, op0=A.mult)
            pm = mk(ZW, nm="pm")
            nc.vector.tensor_tensor(pm[:], qf, tmp[:], op=A.add)
            v2 = mk(ntg, nm="v2")
            nc.vector.tensor_reduce(v2[:], v3(pm[:]), axis=AX.X, op=A.max)
            m2 = mk(ZW, nm="m2")
            nc.vector.tensor_tensor(v3(m2[:]), v3(pm[:]),
                                    v2[:].unsqueeze(2).broadcast_to([128, ntg, 6]),
                                    op=A.is_equal)
            d_ = mk(ntg, nm="d")
            nc.vector.tensor_tensor(d_[:], v2[:], v1[:], op=A.subtract)
            t_ = mk(ntg, nm="t")
            nc.vector.tensor_tensor(t_[:], d_[:], r[:], op=A.mult)
            e2 = mk(ntg, nm="e2")
            nc.scalar.activation(e2[:], t_[:], AF.Exp)
            s2 = mk(ntg, nm="s2")
            nc.vector.tensor_scalar(s2[:], e2[:], 1.0, None, op0=A.add)
            w1_ = mk(ntg, nm="w1")
            nc.vector.reciprocal(w1_[:], s2[:])
            w2_ = mk(ntg, nm="w2")
            nc.vector.tensor_tensor(w2_[:], e2[:], w1_[:], op=A.mult)
            g1 = mk(ZW, nm="g1")
            nc.vector.tensor_tensor(v3(g1[:]), v3(m1[:]),
                                    w1_[:].unsqueeze(2).broadcast_to([128, ntg, 6]),
                                    op=A.mult)
            g = mk(ZW, nm="g")
            nc.vector.tensor_tensor(v3(g[:]), v3(m2[:]),
                                    w2_[:].unsqueeze(2).broadcast_to([128, ntg, 6]),
                                    op=A.mult)
            nc.vector.tensor_tensor(g[:], g[:], g1[:], op=A.add)

            def ved(ap, dd):
                return ap.rearrange("p (t e d) -> p t e d", e=6, d=dd)

            def vde(ap, dd):
                return ap.rearrange("p (t e d) -> p t d e", e=6, d=dd)

            g4 = v3(g[:]).unsqueeze(3).broadcast_to([128, ntg, 6, 4])
            nc.vector.tensor_tensor(ved(db[sgi][:], 4), ved(db[sgi][:], 4),
                                    g4, op=A.mult)
            g6 = v3(g[:]).unsqueeze(3).broadcast_to([128, ntg, 6, 6])
            nc.vector.tensor_tensor(ved(dc[sgi][:], 6), ved(dc[sgi][:], 6),
                                    g6, op=A.mult)
            oall = mk(ntg * 10, nm="o")
            o10 = oall[:].rearrange("p (t f) -> p t f", f=10)
            nc.vector.tensor_reduce(o10[:, :, 0:4], vde(db[sgi][:], 4),
                                    axis=AX.X, op=A.add)
            nc.vector.tensor_reduce(o10[:, :, 4:10], vde(dc[sgi][:], 6),
                                    axis=AX.X, op=A.add)
            nc.gpsimd.dma_start(outd[:, t0 * 10:t1 * 10], oall[:])

        # ---- main chunk loop ----
        dma_sg = {2: [0], 4: [1], 6: [2, 3]}
        for c in range(NCHUNK):
            sgi = sg_of_chunk[c]
            g = c // 2
            W = CHUNKW[c]
            gw = GRPW[g]
            if c % 2 == 0 and g > 0:
                xt = [None, None]
                for k in range(2):
                    t = xp.tile([128, 2048], bf16, tag=f"x{k}", name=f"x{k}g{g}")
                    nc.sync.dma_start(
                        t[:, 0:2 * gw],
                        xpk[k * 128:(k + 1) * 128, g, :, 0:gw])
                    xt[k] = t
                xtiles[g] = xt
            for i in dma_sg.get(c, []):
                t0, t1 = tile_rng[i]
                db[i] = big.tile([128, (t1 - t0) * 24], f32, tag=f"db{i}",
                                 name=f"db{i}")
                nc.gpsimd.dma_start(db[i][:], dbd[:, t0 * 24:t1 * 24])
                dc[i] = big.tile([128, (t1 - t0) * 36], f32, tag=f"dc{i}",
                                 name=f"dc{i}")
                nc.gpsimd.dma_start(dc[i][:], dcd[:, t0 * 36:t1 * 36])

            off = (c % 2) * 512
            ph = php.tile([128, 512], f32, tag="ph", name=f"ph{c}")
            mms = []
            for k in range(2):
                hi = xtiles[g][k][:, off:off + W]
                lo = xtiles[g][k][:, gw + off:gw + off + W]
                mms += [(w1hi[k], hi), (w1hi[k], lo), (w1lo[k], hi)]
            for i, (wt, mt) in enumerate(mms):
                nc.tensor.matmul(ph[:, 0:W], wt, mt,
                                 start=(i == 0), stop=(i == len(mms) - 1))
            hr = hp.tile([128, 512], f32, tag="hr", name=f"hr{c}")
            nc.scalar.activation(hr[:, 0:W], ph[:, 0:W], AF.Relu,
                                 bias=b1t, scale=1.0)

            if c == sg_start[sgi]:
                ztp[sgi] = pzp.tile([128, 512], f32, tag="ztp", name=f"ztp{sgi}")
            cg = c - sg_start[sgi]
            nc.tensor.matmul(ztp[sgi][32 * cg:32 * cg + 32, 0:W],
                             w2t, hr[:, 0:W],
                             start=True, stop=True,
                             tile_position=(0, 32 * cg),
                             skip_group_check=True)

            if c == sg_start[sgi] + SGS[sgi] - 1:
                z = emit_sg_tail(sgi, W)
                emit_phase_b(sgi, z)

    nc.compile()
    return nc


def _get_program():
    global _PROG
    if _PROG is None:
        _PROG = _build_program()
    return _PROG


def _prep_expert(data, dd):
    """[E,B,Q,dd] -> per-core [128, NT*E*dd] in (p, t, e, d) token-tile order."""
    flat = np.ascontiguousarray(np.transpose(data, (1, 2, 0, 3))).reshape(B * Q, E * dd)
    out = []
    for core in range(NCORES):
        sl = flat[core * NTOK:(core + 1) * NTOK]
        pad = np.zeros((NTOKP, E * dd), np.float32)
        pad[:NTOK] = sl
        arr = pad.reshape(NT, 128, E * dd).transpose(1, 0, 2)
        out.append(np.ascontiguousarray(arr.reshape(128, NT * E * dd)))
    return out


def kernel(features, W1, b1, W2, b2, expert_bboxes, expert_class_scores):
    global _LAST_IN_MAPS
    features = np.asarray(features, np.float32)
    W1 = np.asarray(W1, np.float32)
    b1 = np.asarray(b1, np.float32)
    W2 = np.asarray(W2, np.float32)
    b2 = np.asarray(b2, np.float32)
    expert_bboxes = np.asarray(expert_bboxes, np.float32)
    expert_class_scores = np.asarray(expert_class_scores, np.float32)

    w1hi = W1.astype(ml_dtypes.bfloat16)
    w1lo = (W1 - w1hi.astype(np.float32)).astype(ml_dtypes.bfloat16)
    cb16 = np.zeros((128, 512), ml_dtypes.bfloat16)
    for k in range(2):
        cb16[:, k * 256:k * 256 + 128] = w1hi[k * 128:(k + 1) * 128, :]
        cb16[:, k * 256 + 128:k * 256 + 256] = w1lo[k * 128:(k + 1) * 128, :]

    cf32 = np.zeros((128, 167), np.float32)
    cf32[:, 0:E] = W2
    cf32[:, 32] = b1
    cf32[:, 33:161] = np.eye(128, dtype=np.float32)
    cf32[:, 161:167] = b2[None, :]

    cu32 = np.ascontiguousarray(
        np.broadcast_to(5 - np.arange(E, dtype=np.uint32), (128, E)))

    db_all = _prep_expert(expert_bboxes, 4)
    dc_all = _prep_expert(expert_class_scores, C)

    in_maps = []
    for core in range(NCORES):
        b0 = core * BPC
        x = np.ascontiguousarray(
            features[b0:b0 + BPC, :Q, :].reshape(NTOK, D).T)
        xt = np.zeros((D, NTOKP), np.float32)
        xt[:, :NTOK] = x
        xhi = xt.astype(ml_dtypes.bfloat16)
        xlo = (xt - xhi.astype(np.float32)).astype(ml_dtypes.bfloat16)
        xpk = np.zeros((D, NGRP, 2, 1024), ml_dtypes.bfloat16)
        for g in range(NGRP):
            gw = GRPW[g]
            xpk[:, g, 0, :gw] = xhi[:, g * 1024:g * 1024 + gw]
            xpk[:, g, 1, :gw] = xlo[:, g * 1024:g * 1024 + gw]
        in_maps.append({
            "xpk": np.ascontiguousarray(xpk),
            "cb16": cb16, "cf32": cf32, "cu32": cu32,
            "db": db_all[core], "dc": dc_all[core],
        })
    _LAST_IN_MAPS = in_maps

    nc = _get_program()
    from concourse.bass_utils import run_bass_kernel_spmd
    res = run_bass_kernel_spmd(nc, in_maps, list(range(NCORES)))

    bb = np.zeros((B, Q, 4), np.float32)
    cc = np.zeros((B, Q, C), np.float32)
    for core in range(NCORES):
        o = res.results[core]["out"]
        o = o.reshape(128, NT, 10).transpose(1, 0, 2).reshape(NTOKP, 10)[:NTOK]
        b0 = core * BPC
        bb[b0:b0 + BPC] = o[:, 0:4].reshape(BPC, Q, 4)
        cc[b0:b0 + BPC] = o[:, 4:10].reshape(BPC, Q, C)
    return bb, cc
